# revision 4
# baseline (speedup 1.0000x reference)
"""Bahdanau additive attention kernel for Trainium2 (8 NeuronCores) — v3.

Reference computation (B=32, S=4096, D=512):
    pre   = enc @ We.T + (hidden @ Wh.T + b1)[:, None, :]   # [B, S, D]
    h     = tanh(pre)
    e     = h @ w2                                          # [B, S]
    alpha = softmax(e, axis=1)
    ctx   = einsum('bs,bsd->bd', alpha, enc)                # [B, D]

All-bf16 (fp8 anywhere upstream of alpha fails the 2e-2 gate: ctx is a
weighted mean of zero-mean values, so every noise source lands at full
relative strength).

v3 over the baseline:
  - e matmuls are column-group packed: the two 512-wide e rows of an
    s-tile run CONCURRENTLY on PE column groups 0/32 (tile_position),
    halving e-matmul wall time (27 -> ~14 us of PE).
  - The two e rows land on partitions 0/32 of one PSUM bank; exp runs
    as two [1, 512] ACT ops with fused row-sum accumulation (drops the
    32 ACTIVATION_READ_ACCUMULATOR ops' overhead via fewer, direct
    accum slots).
  - Warm-up burst trimmed 24 -> 14 matmuls.
"""

import sys

if "/opt/trn_rl_repo" not in sys.path:
    sys.path.insert(0, "/opt/trn_rl_repo")

from contextlib import ExitStack

import ml_dtypes
import numpy as np

import concourse.bass as bass
import concourse.bacc as bacc
import concourse.tile as tile
from concourse import mybir
from concourse.bass_utils import run_bass_kernel_spmd

B, S, D = 32, 4096, 512
NCORES = 8
BPC = B // NCORES          # batches per core
P = 128                    # partitions
NDC = D // P               # d (contraction) chunks
NKC = D // P               # k (output channel) chunks
ST = 1024                  # s-tile size (PE/ACT/DVE granularity)
NST = S // ST              # s tiles per batch
EST = 512                  # e-row granularity
NER = S // EST             # exp rows per batch

F32 = mybir.dt.float32
BF16 = mybir.dt.bfloat16
AF = mybir.ActivationFunctionType
ALU = mybir.AluOpType


def build_bass():
    nc = bacc.Bacc()

    encT = nc.declare_dram_parameter("encT", [BPC, NST, NDC, P, ST], BF16, isOutput=False)
    weT = nc.declare_dram_parameter("weT", [NDC, P, D], BF16, isOutput=False)
    cT = nc.declare_dram_parameter("cT", [P, NKC, BPC], F32, isOutput=False)
    # w2c[p, ki, m]: column 0 holds w2[ki*128+p], columns 1..15 zero
    w2c = nc.declare_dram_parameter("w2c", [P, NKC, 16], BF16, isOutput=False)
    ctx_out = nc.declare_dram_parameter("ctx", [P, NDC, BPC], F32, isOutput=True)

    with TileKernel(nc) as tk:
        tk.build(encT, weT, cT, w2c, ctx_out)
    nc.finalize()
    return nc


class TileKernel:
    def __init__(self, nc):
        self.nc = nc
        self.stack = ExitStack()
        self.tc = None

    def __enter__(self):
        self.tc = self.stack.enter_context(tile.TileContext(self.nc))
        return self

    def __exit__(self, *exc):
        return self.stack.__exit__(*exc)

    def build(self, encT, weT, cT, w2c, ctx_out):
        nc, tc, ctx = self.nc, self.tc, self.stack

        singles = ctx.enter_context(tc.tile_pool(name="singles", bufs=1))
        encp = ctx.enter_context(tc.tile_pool(name="encp", bufs=2 * NST))
        htp = ctx.enter_context(tc.tile_pool(name="htp", bufs=4))
        abp = ctx.enter_context(tc.tile_pool(name="abp", bufs=8))
        junkp = ctx.enter_context(tc.tile_pool(name="junkp", bufs=3))
        smp = ctx.enter_context(tc.tile_pool(name="smp", bufs=3))
        ctxp = ctx.enter_context(tc.tile_pool(name="ctxp", bufs=2))
        dramp = ctx.enter_context(tc.tile_pool(name="dramp", bufs=2, space="DRAM"))
        prep = ctx.enter_context(tc.tile_pool(name="prep", bufs=3, space="PSUM"))
        ecp = ctx.enter_context(tc.tile_pool(name="ecp", bufs=2, space="PSUM"))

        # ---- load constants ----
        # w_sb rides the sync queue ahead of the enc tiles; the other
        # constants go via the gpsimd queue so they don't delay enc tile 0.
        w_sb = singles.tile([P, NDC, D], BF16)
        nc.sync.dma_start(out=w_sb, in_=weT[:].rearrange("di p k -> p di k"))
        w2_sb = singles.tile([P, NKC, 16], BF16)
        nc.gpsimd.dma_start(out=w2_sb, in_=w2c[:])
        c_sb = singles.tile([P, NKC, BPC], F32)
        nc.gpsimd.dma_start(out=c_sb, in_=cT[:])

        # ---- PE warm-up burst ----
        # ~6 us of dummy matmuls on a zeroed tile (no DMA dependency) so the
        # HAM clock gate reaches 8/8 before real work starts.
        wz = singles.tile([P, D], BF16)
        nc.vector.memzero(wz)
        wpre = prep.tile([P, ST], F32, tag="pre")
        for i in range(14):
            nc.tensor.matmul(
                out=wpre[:, 0:D], lhsT=wz[:, 0:P], rhs=wz,
                start=True, stop=True,
            )
        wjunk = singles.tile([P, 1], F32)
        nc.vector.tensor_copy(out=wjunk, in_=wpre[:, 0:1])

        # ---- main per-batch pipeline ----
        # Softmax runs UNNORMALIZED and streamed: e is bounded (|e| < ~5)
        # so exp needs no max pass; exp(e) rows are computed as soon as the
        # e matmuls land in PSUM, round-trip through DRAM (bf16) to
        # partition-broadcast, and feed the context accumulation while the
        # PE works on later s-tiles. The 1/sum(exp) normalization is
        # applied once to the final [128, NDC] context.
        for b in range(BPC):
            pd = dramp.tile([NER, EST], BF16, tag="pd")
            lparts = smp.tile([64, NST], F32, tag="lparts")
            nc.vector.memzero(lparts)
            cacc = ctxp.tile([P, NDC, NST], F32, tag="cacc")

            def softmax_ctx(st, et, ht):
                # e rows for both halves, packed on PE column groups 0/32:
                # the two accumulation chains run concurrently (separate
                # XBUS streams, separate PSUM banks + partition ranges).
                # Emitted one s-tile behind the main matmuls so the PE
                # never waits on tanh.
                e_ps0 = ecp.tile([16, EST], F32, tag="ec")
                e_ps1 = ecp.tile([48, EST], F32, tag="ec")
                e_pss = [e_ps0, e_ps1]
                for ki in range(NKC):
                    for g in range(ST // EST):
                        sl = slice(g * EST, (g + 1) * EST)
                        nc.tensor.matmul(
                            out=e_pss[g][32 * g:32 * g + 16, :],
                            lhsT=w2_sb[:, ki, :],
                            rhs=ht[:, ki, sl],
                            start=(ki == 0),
                            stop=(ki == NKC - 1),
                            tile_position=(0, 32 * g),
                        )
                # p = exp(e) per row with the row-sum fused; bf16 rows go
                # out through DRAM so DMA can replicate across partitions.
                p_rows = smp.tile([48, EST], BF16, tag="prow")
                for g in range(ST // EST):
                    nc.scalar.activation(
                        out=p_rows[32 * g:32 * g + 1, :],
                        in_=e_pss[g][32 * g:32 * g + 1, :],
                        func=AF.Exp, bias=0.0, scale=1.0,
                        accum_out=lparts[32 * g:32 * g + 1, st:st + 1],
                    )
                for g in range(ST // EST):
                    r = st * (ST // EST) + g
                    nc.scalar.dma_start(
                        out=pd[r:r + 1, :], in_=p_rows[32 * g:32 * g + 1, :])
                # broadcast the two p rows of this s-tile across partitions,
                # split into two half DMAs on separate queues so the
                # replication bandwidth doubles.
                ab = abp.tile([P, ST], BF16, tag="ab")
                for g, issuer in ((0, nc.gpsimd), (1, nc.sync)):
                    r = st * (ST // EST) + g
                    rows = pd[r:r + 1, :]
                    issuer.dma_start(
                        out=ab[:, g * EST:(g + 1) * EST],
                        in_=bass.AP(
                            tensor=rows.tensor,
                            offset=rows.offset,
                            ap=[[0, P], [1, EST]],
                        ),
                    )
                for di in range(NDC):
                    junk = junkp.tile([P, ST], BF16, tag="junk")
                    nc.vector.scalar_tensor_tensor(
                        out=junk,
                        in0=et[:, di, :],
                        scalar=1.0,
                        in1=ab,
                        op0=ALU.mult,
                        op1=ALU.mult,
                        accum_out=cacc[:, di, st:st + 1],
                    )

            prev = None
            for st in range(NST):
                et = encp.tile([P, NDC, ST], BF16, tag="et")
                eng = nc.sync if st % 2 == 0 else nc.scalar
                eng.dma_start(out=et, in_=encT[:][b, st].rearrange("di p s -> p di s"))

                ht = htp.tile([P, NKC, ST], BF16, tag="ht")
                for ki in range(NKC):
                    pre_ps = prep.tile([P, ST], F32, tag="pre")
                    for half in range(ST // EST):
                        sl = slice(half * EST, (half + 1) * EST)
                        for di in range(NDC):
                            nc.tensor.matmul(
                                out=pre_ps[:, sl],
                                lhsT=w_sb[:, di, ki * P:(ki + 1) * P],
                                rhs=et[:, di, sl],
                                start=(di == 0),
                                stop=(di == NDC - 1),
                            )
                    # h^T = tanh(pre^T + c), one [128, ST] ACT op per ki
                    nc.scalar.activation(
                        out=ht[:, ki, :],
                        in_=pre_ps,
                        func=AF.Tanh,
                        bias=c_sb[:, ki, b:b + 1],
                        scale=1.0,
                    )
                if prev is not None:
                    softmax_ctx(*prev)
                prev = (st, et, ht)
            softmax_ctx(*prev)

            # ---- finalize: ctx = (sum_s p*enc) / sum_s p ----
            lsum2 = smp.tile([64, 1], F32, tag="lsum2")
            nc.vector.reduce_sum(out=lsum2, in_=lparts, axis=mybir.AxisListType.X)
            # gather the two per-row partials onto one partition via tiny
            # SBUF->SBUF DMAs (keeps the PE queue out of the softmax path)
            lpair = smp.tile([1, 2], F32, tag="lpair")
            nc.scalar.dma_start(out=lpair[:, 0:1], in_=lsum2[0:1, :])
            nc.scalar.dma_start(out=lpair[:, 1:2], in_=lsum2[32:33, :])
            lsum = smp.tile([1, 1], F32, tag="lsum")
            nc.vector.reduce_sum(out=lsum, in_=lpair, axis=mybir.AxisListType.X)
            rinv1 = smp.tile([1, 1], F32, tag="rinv1")
            nc.vector.reciprocal(out=rinv1, in_=lsum)
            rinvb = smp.tile([P, 1], F32, tag="rinvb")
            nc.gpsimd.partition_broadcast(out_ap=rinvb, in_ap=rinv1)
            ctx_acc = ctxp.tile([P, NDC], F32, tag="ctx")
            nc.vector.reduce_sum(out=ctx_acc, in_=cacc, axis=mybir.AxisListType.X)
            nc.vector.tensor_scalar_mul(out=ctx_acc, in0=ctx_acc, scalar1=rinvb)
            nc.sync.dma_start(out=ctx_out[:][:, :, b], in_=ctx_acc)


_NC_CACHE = None


def _get_nc():
    global _NC_CACHE
    if _NC_CACHE is None:
        _NC_CACHE = build_bass()
    return _NC_CACHE


def _prep_core_inputs(hidden_state, encoder_outputs, W1, b1, w2, core):
    bf16 = ml_dtypes.bfloat16
    b0 = core * BPC
    enc = encoder_outputs[b0:b0 + BPC]                      # [BPC, S, D] f32
    # [b, d, s] -> [b, di, p, s] -> [b, st, di, p, s]
    e = enc.transpose(0, 2, 1).reshape(BPC, NDC, P, NST, ST)
    e = np.ascontiguousarray(e.transpose(0, 3, 1, 2, 4)).astype(bf16)
    w2cv = np.zeros((P, NKC, 16), dtype=np.float32)
    w2cv[:, :, 0] = w2.reshape(NKC, P).T
    return {
        "encT": e,
        "weT": np.ascontiguousarray(W1[:, :D].T.reshape(NDC, P, D)).astype(bf16),
        "cT": np.ascontiguousarray(
            (hidden_state[b0:b0 + BPC] @ W1[:, D:].T + b1).T.reshape(NKC, P, BPC)
            .transpose(1, 0, 2)),
        "w2c": w2cv.astype(bf16),
    }


def kernel(hidden_state, encoder_outputs, W1, b1, w2, _trace=False, _trace_kwargs=None):
    hidden_state = np.asarray(hidden_state, dtype=np.float32)
    encoder_outputs = np.asarray(encoder_outputs, dtype=np.float32)
    W1 = np.asarray(W1, dtype=np.float32)
    b1 = np.asarray(b1, dtype=np.float32)
    w2 = np.asarray(w2, dtype=np.float32)

    nc = _get_nc()
    in_maps = [
        _prep_core_inputs(hidden_state, encoder_outputs, W1, b1, w2, c)
        for c in range(NCORES)
    ]
    res = run_bass_kernel_spmd(
        nc, in_maps, list(range(NCORES)), trace=_trace,
        **(_trace_kwargs or {}),
    )
    out = np.empty((B, D), dtype=np.float32)
    for c in range(NCORES):
        r = res.results[c]["ctx"]                          # [p, di, b]
        out[c * BPC:(c + 1) * BPC] = r.transpose(2, 1, 0).reshape(BPC, D)
    if _trace:
        return out, res
    return out


# revision 5
# speedup vs baseline: 1.0862x; 1.0862x over previous
"""Bahdanau additive attention kernel for Trainium2 (8 NeuronCores) — v3.

Reference computation (B=32, S=4096, D=512):
    pre   = enc @ We.T + (hidden @ Wh.T + b1)[:, None, :]   # [B, S, D]
    h     = tanh(pre)
    e     = h @ w2                                          # [B, S]
    alpha = softmax(e, axis=1)
    ctx   = einsum('bs,bsd->bd', alpha, enc)                # [B, D]

All-bf16 (fp8 anywhere upstream of alpha fails the 2e-2 gate: ctx is a
weighted mean of zero-mean values, so every noise source lands at full
relative strength).

v3 over the baseline:
  - e matmuls are column-group packed: the two 512-wide e rows of an
    s-tile run CONCURRENTLY on PE column groups 0/32 (tile_position),
    halving e-matmul wall time (27 -> ~14 us of PE).
  - The two e rows land on partitions 0/32 of one PSUM bank; exp runs
    as two [1, 512] ACT ops with fused row-sum accumulation (drops the
    32 ACTIVATION_READ_ACCUMULATOR ops' overhead via fewer, direct
    accum slots).
  - Warm-up burst trimmed 24 -> 14 matmuls.
"""

import sys

if "/opt/trn_rl_repo" not in sys.path:
    sys.path.insert(0, "/opt/trn_rl_repo")

from contextlib import ExitStack

import ml_dtypes
import numpy as np

import concourse.bass as bass
import concourse.bacc as bacc
import concourse.tile as tile
from concourse import mybir
from concourse.bass_utils import run_bass_kernel_spmd

B, S, D = 32, 4096, 512
NCORES = 8
BPC = B // NCORES          # batches per core
P = 128                    # partitions
NDC = D // P               # d (contraction) chunks
NKC = D // P               # k (output channel) chunks
ST = 1024                  # max s-tile width (buffer sizing)
EST = 512                  # e-row granularity
NER = S // EST             # exp rows per batch
# s-tile widths per batch: narrow edge tiles shorten the first-tile DMA
# latency (pipeline fill) and the last tile's softmax/context drain.
WIDTHS = [512, 1024, 1024, 1024, 512]
NST = len(WIDTHS)

F32 = mybir.dt.float32
BF16 = mybir.dt.bfloat16
AF = mybir.ActivationFunctionType
ALU = mybir.AluOpType


def build_bass():
    nc = bacc.Bacc()

    encT = nc.declare_dram_parameter("encT", [BPC, NDC, P, S], BF16, isOutput=False)
    weT = nc.declare_dram_parameter("weT", [NDC, P, D], BF16, isOutput=False)
    cT = nc.declare_dram_parameter("cT", [P, NKC, BPC], F32, isOutput=False)
    # w2c[p, ki, m]: column 0 holds w2[ki*128+p], columns 1..15 zero
    w2c = nc.declare_dram_parameter("w2c", [P, NKC, 16], BF16, isOutput=False)
    ctx_out = nc.declare_dram_parameter("ctx", [P, NDC, BPC], F32, isOutput=True)

    with TileKernel(nc) as tk:
        tk.build(encT, weT, cT, w2c, ctx_out)
    nc.finalize()
    return nc


class TileKernel:
    def __init__(self, nc):
        self.nc = nc
        self.stack = ExitStack()
        self.tc = None

    def __enter__(self):
        self.tc = self.stack.enter_context(tile.TileContext(self.nc))
        return self

    def __exit__(self, *exc):
        return self.stack.__exit__(*exc)

    def build(self, encT, weT, cT, w2c, ctx_out):
        nc, tc, ctx = self.nc, self.tc, self.stack

        singles = ctx.enter_context(tc.tile_pool(name="singles", bufs=1))
        encp = ctx.enter_context(tc.tile_pool(name="encp", bufs=2 * NST))
        htp = ctx.enter_context(tc.tile_pool(name="htp", bufs=4))
        abp = ctx.enter_context(tc.tile_pool(name="abp", bufs=8))
        junkp = ctx.enter_context(tc.tile_pool(name="junkp", bufs=3))
        smp = ctx.enter_context(tc.tile_pool(name="smp", bufs=3))
        ctxp = ctx.enter_context(tc.tile_pool(name="ctxp", bufs=2))
        dramp = ctx.enter_context(tc.tile_pool(name="dramp", bufs=2, space="DRAM"))
        prep = ctx.enter_context(tc.tile_pool(name="prep", bufs=3, space="PSUM"))
        ecp = ctx.enter_context(tc.tile_pool(name="ecp", bufs=2, space="PSUM"))

        # ---- load constants ----
        # w_sb rides the sync queue ahead of the enc tiles; the other
        # constants go via the gpsimd queue so they don't delay enc tile 0.
        w_sb = singles.tile([P, NDC, D], BF16)
        nc.gpsimd.dma_start(out=w_sb, in_=weT[:].rearrange("di p k -> p di k"))
        w2_sb = singles.tile([P, NKC, 16], BF16)
        nc.gpsimd.dma_start(out=w2_sb, in_=w2c[:])
        c_sb = singles.tile([P, NKC, BPC], F32)
        nc.gpsimd.dma_start(out=c_sb, in_=cT[:])

        # ---- PE warm-up burst ----
        # ~6 us of dummy matmuls on a zeroed tile (no DMA dependency) so the
        # HAM clock gate reaches 8/8 before real work starts.
        wz = singles.tile([P, D], BF16)
        nc.vector.memzero(wz)
        wpre = prep.tile([P, ST], F32, tag="pre")
        for i in range(14):
            nc.tensor.matmul(
                out=wpre[:, 0:D], lhsT=wz[:, 0:P], rhs=wz,
                start=True, stop=True,
            )
        wjunk = singles.tile([P, 1], F32)
        nc.vector.tensor_copy(out=wjunk, in_=wpre[:, 0:1])

        # ---- main per-batch pipeline ----
        # Softmax runs UNNORMALIZED and streamed: e is bounded (|e| < ~5)
        # so exp needs no max pass; exp(e) rows are computed as soon as the
        # e matmuls land in PSUM, round-trip through DRAM (bf16) to
        # partition-broadcast, and feed the context accumulation while the
        # PE works on later s-tiles. The 1/sum(exp) normalization is
        # applied once to the final [128, NDC] context.
        for b in range(BPC):
            pd = dramp.tile([NER, EST], BF16, tag="pd")
            lparts = smp.tile([64, NST], F32, tag="lparts")
            nc.vector.memzero(lparts)
            cacc = ctxp.tile([P, NDC, NST], F32, tag="cacc")

            def softmax_ctx(t, w, r0, et, ht):
                ng = w // EST
                # e rows, packed on PE column groups 0/32: the (up to) two
                # accumulation chains run concurrently (separate XBUS
                # streams, separate PSUM banks + partition ranges). Emitted
                # one s-tile behind the main matmuls so the PE never waits
                # on tanh.
                e_ps0 = ecp.tile([16, EST], F32, tag="ec")
                e_ps1 = ecp.tile([48, EST], F32, tag="ec")
                e_pss = [e_ps0, e_ps1]
                for ki in range(NKC):
                    for g in range(ng):
                        sl = slice(g * EST, (g + 1) * EST)
                        nc.tensor.matmul(
                            out=e_pss[g][32 * g:32 * g + 16, :],
                            lhsT=w2_sb[:, ki, :],
                            rhs=ht[:, ki, sl],
                            start=(ki == 0),
                            stop=(ki == NKC - 1),
                            tile_position=(0, 32 * g),
                        )
                # p = exp(e) per row with the row-sum fused; bf16 rows go
                # out through DRAM so DMA can replicate across partitions.
                p_rows = smp.tile([48, EST], BF16, tag="prow")
                for g in range(ng):
                    nc.scalar.activation(
                        out=p_rows[32 * g:32 * g + 1, :],
                        in_=e_pss[g][32 * g:32 * g + 1, :],
                        func=AF.Exp, bias=0.0, scale=1.0,
                        accum_out=lparts[32 * g:32 * g + 1, t:t + 1],
                    )
                for g in range(ng):
                    nc.sync.dma_start(
                        out=pd[r0 + g:r0 + g + 1, :],
                        in_=p_rows[32 * g:32 * g + 1, :])
                # broadcast the p rows across partitions, one DMA per
                # 512-wide row on alternating queues.
                ab = abp.tile([P, ST], BF16, tag="ab")
                for g, issuer in zip(range(ng), (nc.gpsimd, nc.sync)):
                    rows = pd[r0 + g:r0 + g + 1, :]
                    issuer.dma_start(
                        out=ab[:, g * EST:(g + 1) * EST],
                        in_=bass.AP(
                            tensor=rows.tensor,
                            offset=rows.offset,
                            ap=[[0, P], [1, EST]],
                        ),
                    )
                for di in range(NDC):
                    junk = junkp.tile([P, ST], BF16, tag="junk")
                    nc.vector.scalar_tensor_tensor(
                        out=junk[:, 0:w],
                        in0=et[:, di, 0:w],
                        scalar=1.0,
                        in1=ab[:, 0:w],
                        op0=ALU.mult,
                        op1=ALU.mult,
                        accum_out=cacc[:, di, t:t + 1],
                    )

            prev = None
            s0 = 0
            for t, w in enumerate(WIDTHS):
                et = encp.tile([P, NDC, ST], BF16, tag="et")
                eng = nc.sync if t % 2 == 0 else nc.gpsimd
                eng.dma_start(
                    out=et[:, :, 0:w],
                    in_=encT[:][b, :, :, s0:s0 + w].rearrange("di p s -> p di s"))

                ht = htp.tile([P, NKC, ST], BF16, tag="ht")
                for ki in range(NKC):
                    pre_ps = prep.tile([P, ST], F32, tag="pre")
                    for half in range(w // EST):
                        sl = slice(half * EST, (half + 1) * EST)
                        for di in range(NDC):
                            nc.tensor.matmul(
                                out=pre_ps[:, sl],
                                lhsT=w_sb[:, di, ki * P:(ki + 1) * P],
                                rhs=et[:, di, sl],
                                start=(di == 0),
                                stop=(di == NDC - 1),
                            )
                    # h^T = tanh(pre^T + c), one [128, w] ACT op per ki
                    nc.scalar.activation(
                        out=ht[:, ki, 0:w],
                        in_=pre_ps[:, 0:w],
                        func=AF.Tanh,
                        bias=c_sb[:, ki, b:b + 1],
                        scale=1.0,
                    )
                if prev is not None:
                    softmax_ctx(*prev)
                prev = (t, w, s0 // EST, et, ht)
                s0 += w
            softmax_ctx(*prev)

            # ---- finalize: ctx = (sum_s p*enc) / sum_s p ----
            lsum2 = smp.tile([64, 1], F32, tag="lsum2")
            nc.vector.reduce_sum(out=lsum2, in_=lparts, axis=mybir.AxisListType.X)
            # gather the two per-row partials onto one partition via tiny
            # SBUF->SBUF DMAs (keeps the PE queue out of the softmax path)
            lpair = smp.tile([1, 2], F32, tag="lpair")
            nc.gpsimd.dma_start(out=lpair[:, 0:1], in_=lsum2[0:1, :])
            nc.gpsimd.dma_start(out=lpair[:, 1:2], in_=lsum2[32:33, :])
            lsum = smp.tile([1, 1], F32, tag="lsum")
            nc.vector.reduce_sum(out=lsum, in_=lpair, axis=mybir.AxisListType.X)
            rinv1 = smp.tile([1, 1], F32, tag="rinv1")
            nc.vector.reciprocal(out=rinv1, in_=lsum)
            rinvb = smp.tile([P, 1], F32, tag="rinvb")
            nc.gpsimd.partition_broadcast(out_ap=rinvb, in_ap=rinv1)
            ctx_acc = ctxp.tile([P, NDC], F32, tag="ctx")
            nc.vector.reduce_sum(out=ctx_acc, in_=cacc, axis=mybir.AxisListType.X)
            nc.vector.tensor_scalar_mul(out=ctx_acc, in0=ctx_acc, scalar1=rinvb)
            nc.sync.dma_start(out=ctx_out[:][:, :, b], in_=ctx_acc)


_NC_CACHE = None


def _get_nc():
    global _NC_CACHE
    if _NC_CACHE is None:
        _NC_CACHE = build_bass()
    return _NC_CACHE


def _prep_core_inputs(hidden_state, encoder_outputs, W1, b1, w2, core):
    bf16 = ml_dtypes.bfloat16
    b0 = core * BPC
    enc = encoder_outputs[b0:b0 + BPC]                      # [BPC, S, D] f32
    # [b, d, s] -> [b, di, p, s] flat along s
    e = enc.transpose(0, 2, 1).reshape(BPC, NDC, P, S).astype(bf16)
    e = np.ascontiguousarray(e)
    w2cv = np.zeros((P, NKC, 16), dtype=np.float32)
    w2cv[:, :, 0] = w2.reshape(NKC, P).T
    return {
        "encT": e,
        "weT": np.ascontiguousarray(W1[:, :D].T.reshape(NDC, P, D)).astype(bf16),
        "cT": np.ascontiguousarray(
            (hidden_state[b0:b0 + BPC] @ W1[:, D:].T + b1).T.reshape(NKC, P, BPC)
            .transpose(1, 0, 2)),
        "w2c": w2cv.astype(bf16),
    }


def kernel(hidden_state, encoder_outputs, W1, b1, w2, _trace=False, _trace_kwargs=None):
    hidden_state = np.asarray(hidden_state, dtype=np.float32)
    encoder_outputs = np.asarray(encoder_outputs, dtype=np.float32)
    W1 = np.asarray(W1, dtype=np.float32)
    b1 = np.asarray(b1, dtype=np.float32)
    w2 = np.asarray(w2, dtype=np.float32)

    nc = _get_nc()
    in_maps = [
        _prep_core_inputs(hidden_state, encoder_outputs, W1, b1, w2, c)
        for c in range(NCORES)
    ]
    res = run_bass_kernel_spmd(
        nc, in_maps, list(range(NCORES)), trace=_trace,
        **(_trace_kwargs or {}),
    )
    out = np.empty((B, D), dtype=np.float32)
    for c in range(NCORES):
        r = res.results[c]["ctx"]                          # [p, di, b]
        out[c * BPC:(c + 1) * BPC] = r.transpose(2, 1, 0).reshape(BPC, D)
    if _trace:
        return out, res
    return out


# revision 6
# speedup vs baseline: 1.1454x; 1.0544x over previous
"""Bahdanau additive attention kernel for Trainium2 (8 NeuronCores) — v3.

Reference computation (B=32, S=4096, D=512):
    pre   = enc @ We.T + (hidden @ Wh.T + b1)[:, None, :]   # [B, S, D]
    h     = tanh(pre)
    e     = h @ w2                                          # [B, S]
    alpha = softmax(e, axis=1)
    ctx   = einsum('bs,bsd->bd', alpha, enc)                # [B, D]

All-bf16 (fp8 anywhere upstream of alpha fails the 2e-2 gate: ctx is a
weighted mean of zero-mean values, so every noise source lands at full
relative strength).

v3 over the baseline:
  - e matmuls are column-group packed: the two 512-wide e rows of an
    s-tile run CONCURRENTLY on PE column groups 0/32 (tile_position),
    halving e-matmul wall time (27 -> ~14 us of PE).
  - The two e rows land on partitions 0/32 of one PSUM bank; exp runs
    as two [1, 512] ACT ops with fused row-sum accumulation (drops the
    32 ACTIVATION_READ_ACCUMULATOR ops' overhead via fewer, direct
    accum slots).
  - Warm-up burst trimmed 24 -> 14 matmuls.
"""

import sys

if "/opt/trn_rl_repo" not in sys.path:
    sys.path.insert(0, "/opt/trn_rl_repo")

from contextlib import ExitStack

import ml_dtypes
import numpy as np

import concourse.bass as bass
import concourse.bacc as bacc
import concourse.tile as tile
from concourse import mybir
from concourse.bass_utils import run_bass_kernel_spmd

B, S, D = 32, 4096, 512
NCORES = 8
BPC = B // NCORES          # batches per core
P = 128                    # partitions
NDC = D // P               # d (contraction) chunks
NKC = D // P               # k (output channel) chunks
ST = 1024                  # max s-tile width (buffer sizing)
EST = 512                  # e-row granularity
NER = S // EST             # exp rows per batch
# s-tile widths per batch: a narrow first tile on batch 0 shortens the
# pipeline-fill DMA latency; narrow last tiles on the final batch shorten
# the softmax/context drain after the PE finishes.
BATCH_WIDTHS = [
    [512, 1024, 1024, 1024, 512],
    [1024, 1024, 1024, 1024],
    [1024, 1024, 1024, 1024],
    [1024, 1024, 1024, 512, 512],
]
NST = max(len(w) for w in BATCH_WIDTHS)

F32 = mybir.dt.float32
BF16 = mybir.dt.bfloat16
AF = mybir.ActivationFunctionType
ALU = mybir.AluOpType


def build_bass():
    nc = bacc.Bacc()

    encT = nc.declare_dram_parameter("encT", [BPC, NDC, P, S], BF16, isOutput=False)
    weT = nc.declare_dram_parameter("weT", [NDC, P, D], BF16, isOutput=False)
    cT = nc.declare_dram_parameter("cT", [P, NKC, BPC], F32, isOutput=False)
    # w2c[p, ki, m]: column 0 holds w2[ki*128+p], columns 1..15 zero
    w2c = nc.declare_dram_parameter("w2c", [P, NKC, 16], BF16, isOutput=False)
    ctx_out = nc.declare_dram_parameter("ctx", [P, NDC, BPC], F32, isOutput=True)

    with TileKernel(nc) as tk:
        tk.build(encT, weT, cT, w2c, ctx_out)
    nc.finalize()
    return nc


class TileKernel:
    def __init__(self, nc):
        self.nc = nc
        self.stack = ExitStack()
        self.tc = None

    def __enter__(self):
        self.tc = self.stack.enter_context(tile.TileContext(self.nc))
        return self

    def __exit__(self, *exc):
        return self.stack.__exit__(*exc)

    def build(self, encT, weT, cT, w2c, ctx_out):
        nc, tc, ctx = self.nc, self.tc, self.stack

        singles = ctx.enter_context(tc.tile_pool(name="singles", bufs=1))
        encp = ctx.enter_context(tc.tile_pool(name="encp", bufs=2 * NST))
        htp = ctx.enter_context(tc.tile_pool(name="htp", bufs=4))
        abp = ctx.enter_context(tc.tile_pool(name="abp", bufs=8))
        junkp = ctx.enter_context(tc.tile_pool(name="junkp", bufs=3))
        smp = ctx.enter_context(tc.tile_pool(name="smp", bufs=3))
        ctxp = ctx.enter_context(tc.tile_pool(name="ctxp", bufs=2))
        dramp = ctx.enter_context(tc.tile_pool(name="dramp", bufs=2, space="DRAM"))
        prep = ctx.enter_context(tc.tile_pool(name="prep", bufs=3, space="PSUM"))
        ecp = ctx.enter_context(tc.tile_pool(name="ecp", bufs=2, space="PSUM"))

        # ---- load constants ----
        # w_sb rides the sync queue ahead of the enc tiles; the other
        # constants go via the gpsimd queue so they don't delay enc tile 0.
        w_sb = singles.tile([P, NDC, D], BF16)
        nc.gpsimd.dma_start(out=w_sb, in_=weT[:].rearrange("di p k -> p di k"))
        w2_sb = singles.tile([P, NKC, 16], BF16)
        nc.gpsimd.dma_start(out=w2_sb, in_=w2c[:])
        c_sb = singles.tile([P, NKC, BPC], F32)
        nc.gpsimd.dma_start(out=c_sb, in_=cT[:])

        # ---- PE warm-up burst ----
        # ~6 us of dummy matmuls on a zeroed tile (no DMA dependency) so the
        # HAM clock gate reaches 8/8 before real work starts.
        wz = singles.tile([P, D], BF16)
        nc.vector.memzero(wz)
        wpre = prep.tile([P, ST], F32, tag="pre")
        for i in range(14):
            nc.tensor.matmul(
                out=wpre[:, 0:D], lhsT=wz[:, 0:P], rhs=wz,
                start=True, stop=True,
            )
        wjunk = singles.tile([P, 1], F32)
        nc.vector.tensor_copy(out=wjunk, in_=wpre[:, 0:1])

        # ---- main per-batch pipeline ----
        # Softmax runs UNNORMALIZED and streamed: e is bounded (|e| < ~5)
        # so exp needs no max pass; exp(e) rows are computed as soon as the
        # e matmuls land in PSUM, round-trip through DRAM (bf16) to
        # partition-broadcast, and feed the context accumulation while the
        # PE works on later s-tiles. The 1/sum(exp) normalization is
        # applied once to the final [128, NDC] context.
        for b in range(BPC):
            nbt = len(BATCH_WIDTHS[b])
            pd = dramp.tile([NER, EST], BF16, tag="pd")
            lparts = smp.tile([64, NST], F32, tag="lparts")
            nc.vector.memzero(lparts)
            cacc = ctxp.tile([P, NDC, nbt], F32, tag="cacc")

            def softmax_ctx(t, w, r0, et, ht):
                ng = w // EST
                # e rows, packed on PE column groups 0/32: the (up to) two
                # accumulation chains run concurrently (separate XBUS
                # streams, separate PSUM banks + partition ranges). Emitted
                # one s-tile behind the main matmuls so the PE never waits
                # on tanh.
                e_ps0 = ecp.tile([16, EST], F32, tag="ec")
                e_ps1 = ecp.tile([48, EST], F32, tag="ec")
                e_pss = [e_ps0, e_ps1]
                for ki in range(NKC):
                    for g in range(ng):
                        sl = slice(g * EST, (g + 1) * EST)
                        nc.tensor.matmul(
                            out=e_pss[g][32 * g:32 * g + 16, :],
                            lhsT=w2_sb[:, ki, :],
                            rhs=ht[:, ki, sl],
                            start=(ki == 0),
                            stop=(ki == NKC - 1),
                            tile_position=(0, 32 * g),
                        )
                # p = exp(e) per row with the row-sum fused; bf16 rows go
                # out through DRAM so DMA can replicate across partitions.
                p_rows = smp.tile([48, EST], BF16, tag="prow")
                for g in range(ng):
                    nc.scalar.activation(
                        out=p_rows[32 * g:32 * g + 1, :],
                        in_=e_pss[g][32 * g:32 * g + 1, :],
                        func=AF.Exp, bias=0.0, scale=1.0,
                        accum_out=lparts[32 * g:32 * g + 1, t:t + 1],
                    )
                for g in range(ng):
                    nc.gpsimd.dma_start(
                        out=pd[r0 + g:r0 + g + 1, :],
                        in_=p_rows[32 * g:32 * g + 1, :])
                # broadcast the p rows across partitions, one DMA per
                # 512-wide row on alternating queues.
                ab = abp.tile([P, ST], BF16, tag="ab")
                for g, issuer in zip(range(ng), (nc.gpsimd, nc.gpsimd)):
                    rows = pd[r0 + g:r0 + g + 1, :]
                    issuer.dma_start(
                        out=ab[:, g * EST:(g + 1) * EST],
                        in_=bass.AP(
                            tensor=rows.tensor,
                            offset=rows.offset,
                            ap=[[0, P], [1, EST]],
                        ),
                    )
                for di in range(NDC):
                    junk = junkp.tile([P, ST], BF16, tag="junk")
                    nc.vector.scalar_tensor_tensor(
                        out=junk[:, 0:w],
                        in0=et[:, di, 0:w],
                        scalar=1.0,
                        in1=ab[:, 0:w],
                        op0=ALU.mult,
                        op1=ALU.mult,
                        accum_out=cacc[:, di, t:t + 1],
                    )

            prev = None
            s0 = 0
            for t, w in enumerate(BATCH_WIDTHS[b]):
                et = encp.tile([P, NDC, ST], BF16, tag="et")
                nc.sync.dma_start(
                    out=et[:, :, 0:w],
                    in_=encT[:][b, :, :, s0:s0 + w].rearrange("di p s -> p di s"))

                ht = htp.tile([P, NKC, ST], BF16, tag="ht")
                for ki in range(NKC):
                    pre_ps = prep.tile([P, ST], F32, tag="pre")
                    for half in range(w // EST):
                        sl = slice(half * EST, (half + 1) * EST)
                        for di in range(NDC):
                            nc.tensor.matmul(
                                out=pre_ps[:, sl],
                                lhsT=w_sb[:, di, ki * P:(ki + 1) * P],
                                rhs=et[:, di, sl],
                                start=(di == 0),
                                stop=(di == NDC - 1),
                            )
                    # h^T = tanh(pre^T + c), one [128, w] ACT op per ki
                    nc.scalar.activation(
                        out=ht[:, ki, 0:w],
                        in_=pre_ps[:, 0:w],
                        func=AF.Tanh,
                        bias=c_sb[:, ki, b:b + 1],
                        scale=1.0,
                    )
                    # half-tile software pipelining: the previous tile's
                    # e/softmax/context chain slots in after this tile's
                    # first ki group, when its tanh inputs are ready but
                    # well before the PE would stall on them.
                    if ki == 0 and prev is not None:
                        softmax_ctx(*prev)
                        prev = None
                prev = (t, w, s0 // EST, et, ht)
                s0 += w
            softmax_ctx(*prev)

            # ---- finalize: ctx = (sum_s p*enc) / sum_s p ----
            lsum2 = smp.tile([64, 1], F32, tag="lsum2")
            nc.vector.reduce_sum(out=lsum2, in_=lparts, axis=mybir.AxisListType.X)
            # gather the two per-row partials onto one partition via tiny
            # SBUF->SBUF DMAs (keeps the PE queue out of the softmax path)
            lpair = smp.tile([1, 2], F32, tag="lpair")
            nc.gpsimd.dma_start(out=lpair[:, 0:1], in_=lsum2[0:1, :])
            nc.gpsimd.dma_start(out=lpair[:, 1:2], in_=lsum2[32:33, :])
            lsum = smp.tile([1, 1], F32, tag="lsum")
            nc.vector.reduce_sum(out=lsum, in_=lpair, axis=mybir.AxisListType.X)
            rinv1 = smp.tile([1, 1], F32, tag="rinv1")
            nc.vector.reciprocal(out=rinv1, in_=lsum)
            rinvb = smp.tile([P, 1], F32, tag="rinvb")
            nc.gpsimd.partition_broadcast(out_ap=rinvb, in_ap=rinv1)
            ctx_acc = ctxp.tile([P, NDC], F32, tag="ctx")
            nc.vector.reduce_sum(out=ctx_acc, in_=cacc, axis=mybir.AxisListType.X)
            nc.vector.tensor_scalar_mul(out=ctx_acc, in0=ctx_acc, scalar1=rinvb)
            nc.sync.dma_start(out=ctx_out[:][:, :, b], in_=ctx_acc)


_NC_CACHE = None


def _get_nc():
    global _NC_CACHE
    if _NC_CACHE is None:
        _NC_CACHE = build_bass()
    return _NC_CACHE


def _prep_core_inputs(hidden_state, encoder_outputs, W1, b1, w2, core):
    bf16 = ml_dtypes.bfloat16
    b0 = core * BPC
    enc = encoder_outputs[b0:b0 + BPC]                      # [BPC, S, D] f32
    # [b, d, s] -> [b, di, p, s] flat along s
    e = enc.transpose(0, 2, 1).reshape(BPC, NDC, P, S).astype(bf16)
    e = np.ascontiguousarray(e)
    w2cv = np.zeros((P, NKC, 16), dtype=np.float32)
    w2cv[:, :, 0] = w2.reshape(NKC, P).T
    return {
        "encT": e,
        "weT": np.ascontiguousarray(W1[:, :D].T.reshape(NDC, P, D)).astype(bf16),
        "cT": np.ascontiguousarray(
            (hidden_state[b0:b0 + BPC] @ W1[:, D:].T + b1).T.reshape(NKC, P, BPC)
            .transpose(1, 0, 2)),
        "w2c": w2cv.astype(bf16),
    }


def kernel(hidden_state, encoder_outputs, W1, b1, w2, _trace=False, _trace_kwargs=None):
    hidden_state = np.asarray(hidden_state, dtype=np.float32)
    encoder_outputs = np.asarray(encoder_outputs, dtype=np.float32)
    W1 = np.asarray(W1, dtype=np.float32)
    b1 = np.asarray(b1, dtype=np.float32)
    w2 = np.asarray(w2, dtype=np.float32)

    nc = _get_nc()
    in_maps = [
        _prep_core_inputs(hidden_state, encoder_outputs, W1, b1, w2, c)
        for c in range(NCORES)
    ]
    res = run_bass_kernel_spmd(
        nc, in_maps, list(range(NCORES)), trace=_trace,
        **(_trace_kwargs or {}),
    )
    out = np.empty((B, D), dtype=np.float32)
    for c in range(NCORES):
        r = res.results[c]["ctx"]                          # [p, di, b]
        out[c * BPC:(c + 1) * BPC] = r.transpose(2, 1, 0).reshape(BPC, D)
    if _trace:
        return out, res
    return out


# revision 12
# speedup vs baseline: 1.2029x; 1.0502x over previous
"""Bahdanau additive attention kernel for Trainium2 (8 NeuronCores).

Reference computation (B=32, S=4096, D=512):
    pre   = enc @ We.T + (hidden @ Wh.T + b1)[:, None, :]   # [B, S, D]
    h     = tanh(pre)
    e     = h @ w2                                          # [B, S]
    alpha = softmax(e, axis=1)
    ctx   = einsum('bs,bsd->bd', alpha, enc)                # [B, D]

Data-parallel over batch (4 batches per core); enc is re-laid-out on host
as [b, di, p, s] bf16 so the contraction dim d sits on SBUF partitions.

Everything stays bf16: fp8 anywhere upstream of alpha fails the accuracy
gate, because ctx is a softmax-weighted mean of zero-mean vectors - the
signal shrinks exactly as fast as independent noise averages out, so any
per-element quantization error lands on ctx at full relative strength
(measured: fp8 enc => 2.7e-2, fp8 main matmul => 2.2e-2 rel err).

Schedule (per core, vs. the naive version):
  - The tiny bias vector c = hidden @ Wh.T + b1 is computed on host
    (1 MFLOP) and rides the ACT bias operand, removing a 1 MB fp32
    weight DMA and 16 fp32 PE matmuls from the critical startup path.
  - PE warm-up burst on a memzero'd tile (no DMA dependency) brings the
    HAM clock gate to 8/8 while the first enc tile streams in.
  - e matmuls are column-group packed (tile_position=(0, 0)/(0, 32)):
    the two 512-wide e rows of an s-tile accumulate CONCURRENTLY on
    separate PE column groups / XBUS streams / PSUM banks, halving their
    PE cost; the zero-padded w2 column trick gives M=16 outputs of which
    row 0 is live.
  - Half-tile software pipelining: tile t's e/softmax/context chain is
    emitted after tile t+1's first ki group, so the in-order PE queue
    never waits on tanh, while the alpha round-trip latency stays ~2 us.
  - exp runs with fused row-sum accumulation (unnormalized streaming
    softmax: |e| < ~5 so no max pass); the two per-batch row-sum
    partials land on partitions 0/32 and are gathered by tiny DMAs, and
    1/sum scales the final context once per batch.
  - alpha rows round-trip through DRAM (bf16) and partition-broadcast
    back as [128, w] tiles feeding the DVE context accumulation
    (scalar_tensor_tensor with fused accum along s).
  - Strict DMA queue roles: sync carries ONLY enc tiles (the PE-feeding
    stream is never queued behind softmax-dependent writes); gpsimd
    carries the softmax-side chain (p rows -> broadcast -> lsum gather)
    in natural dependency order.
  - Variable s-tile widths: batch 0 opens with a 512-wide tile (halves
    the pipeline-fill DMA latency), the last batch closes with two
    512-wide tiles (halves the post-PE softmax/context drain).
"""

import sys

if "/opt/trn_rl_repo" not in sys.path:
    sys.path.insert(0, "/opt/trn_rl_repo")

from contextlib import ExitStack

import ml_dtypes
import numpy as np

import concourse.bass as bass
import concourse.bacc as bacc
from concourse import bass_isa
import concourse.tile as tile
from concourse import mybir
from concourse.bass_utils import run_bass_kernel_spmd

B, S, D = 32, 4096, 512
NCORES = 8
BPC = B // NCORES          # batches per core
P = 128                    # partitions
NDC = D // P               # d (contraction) chunks
NKC = D // P               # k (output channel) chunks
ST = 1024                  # max s-tile width (buffer sizing)
EST = 512                  # e-row granularity
NER = S // EST             # exp rows per batch
# s-tile widths per batch: a narrow first tile on batch 0 shortens the
# pipeline-fill DMA latency; narrow last tiles on the final batch shorten
# the softmax/context drain after the PE finishes.
BATCH_WIDTHS = [
    [512, 1024, 1024, 1024, 512],
    [1024, 1024, 1024, 1024],
    [1024, 1024, 1024, 1024],
    [1024, 1024, 1024, 512, 512],
]
NST = max(len(w) for w in BATCH_WIDTHS)

F32 = mybir.dt.float32
BF16 = mybir.dt.bfloat16
AF = mybir.ActivationFunctionType
ALU = mybir.AluOpType


def build_bass():
    nc = bacc.Bacc()

    encT = nc.declare_dram_parameter("encT", [BPC, NDC, P, S], BF16, isOutput=False)
    weT = nc.declare_dram_parameter("weT", [NDC, P, D], BF16, isOutput=False)
    cT = nc.declare_dram_parameter("cT", [P, NKC, BPC], F32, isOutput=False)
    # w2c[p, ki, m]: column 0 holds w2[ki*128+p], columns 1..15 zero
    w2c = nc.declare_dram_parameter("w2c", [P, NKC, 16], BF16, isOutput=False)
    ctx_out = nc.declare_dram_parameter("ctx", [P, NDC, BPC], F32, isOutput=True)

    with TileKernel(nc) as tk:
        tk.build(encT, weT, cT, w2c, ctx_out)
    nc.finalize()
    return nc


class TileKernel:
    def __init__(self, nc):
        self.nc = nc
        self.stack = ExitStack()
        self.tc = None

    def __enter__(self):
        self.tc = self.stack.enter_context(tile.TileContext(self.nc))
        return self

    def __exit__(self, *exc):
        return self.stack.__exit__(*exc)

    def build(self, encT, weT, cT, w2c, ctx_out):
        nc, tc, ctx = self.nc, self.tc, self.stack

        singles = ctx.enter_context(tc.tile_pool(name="singles", bufs=1))
        encp = ctx.enter_context(tc.tile_pool(name="encp", bufs=2 * NST))
        htp = ctx.enter_context(tc.tile_pool(name="htp", bufs=4))
        abp = ctx.enter_context(tc.tile_pool(name="abp", bufs=8))
        junkp = ctx.enter_context(tc.tile_pool(name="junkp", bufs=3))
        smp = ctx.enter_context(tc.tile_pool(name="smp", bufs=3))
        ctxp = ctx.enter_context(tc.tile_pool(name="ctxp", bufs=2))
        prep = ctx.enter_context(tc.tile_pool(name="prep", bufs=3, space="PSUM"))
        ecp = ctx.enter_context(tc.tile_pool(name="ecp", bufs=2, space="PSUM"))

        # ---- load constants ----
        # w_sb rides the sync queue ahead of the enc tiles; the other
        # constants go via the gpsimd queue so they don't delay enc tile 0.
        w_sb = singles.tile([P, NDC, D], BF16)
        nc.scalar.dma_start(out=w_sb, in_=weT[:].rearrange("di p k -> p di k"))
        w2_sb = singles.tile([P, NKC, 16], BF16)
        nc.scalar.dma_start(out=w2_sb, in_=w2c[:])
        c_sb = singles.tile([P, NKC, BPC], F32)
        nc.scalar.dma_start(out=c_sb, in_=cT[:])

        # ---- PE warm-up burst ----
        # ~6 us of dummy matmuls on a zeroed tile (no DMA dependency) so the
        # HAM clock gate reaches 8/8 before real work starts.
        wz = singles.tile([P, D], BF16)
        nc.vector.memzero(wz)
        wpre = prep.tile([P, ST], F32, tag="pre")
        for i in range(14):
            nc.tensor.matmul(
                out=wpre[:, 0:D], lhsT=wz[:, 0:P], rhs=wz,
                start=True, stop=True,
            )
        wjunk = singles.tile([P, 1], F32)
        nc.vector.tensor_copy(out=wjunk, in_=wpre[:, 0:1])

        # ---- main per-batch pipeline ----
        # Softmax runs UNNORMALIZED and streamed: e is bounded (|e| < ~5)
        # so exp needs no max pass; exp(e) rows are computed as soon as the
        # e matmuls land in PSUM, round-trip through DRAM (bf16) to
        # partition-broadcast, and feed the context accumulation while the
        # PE works on later s-tiles. The 1/sum(exp) normalization is
        # applied once to the final [128, NDC] context.
        for b in range(BPC):
            nbt = len(BATCH_WIDTHS[b])
            lparts = smp.tile([P, NST], F32, tag="lparts")
            nc.vector.memzero(lparts)
            cacc = ctxp.tile([P, NDC, NER], F32, tag="cacc")

            def softmax_ctx(t, w, r0, et, ht):
                ng = w // EST
                # e rows, packed on PE column groups 0/32: the (up to) two
                # accumulation chains run concurrently (separate XBUS
                # streams, separate PSUM banks + partition ranges). Emitted
                # one s-tile behind the main matmuls so the PE never waits
                # on tanh.
                e_ps0 = ecp.tile([16, EST], F32, tag="ec")
                e_ps1 = ecp.tile([48, EST], F32, tag="ec")
                e_pss = [e_ps0, e_ps1]
                for ki in range(NKC):
                    for g in range(ng):
                        sl = slice(g * EST, (g + 1) * EST)
                        nc.tensor.matmul(
                            out=e_pss[g][32 * g:32 * g + 16, :],
                            lhsT=w2_sb[:, ki, :],
                            rhs=ht[:, ki, sl],
                            start=(ki == 0),
                            stop=(ki == NKC - 1),
                            tile_position=(0, 32 * g),
                        )
                # p = exp(e) per row with the row-sum fused; bf16 rows go
                # out through DRAM so DMA can replicate across partitions.
                p_rows = smp.tile([48, EST], BF16, tag="prow")
                for g in range(ng):
                    nc.scalar.activation(
                        out=p_rows[32 * g:32 * g + 1, :],
                        in_=e_pss[g][32 * g:32 * g + 1, :],
                        func=AF.Exp, bias=0.0, scale=1.0,
                        accum_out=lparts[32 * g:32 * g + 1, t:t + 1],
                    )
                # replicate the p rows across partitions with gpsimd
                # partition_broadcast compute ops: no DRAM round trip, and
                # gpsimd stays DMA-free so its SWDGE drain vanishes from
                # the epilogue. The broadcast gets canonical APs only:
                # full-tile outputs, and a partition-0/offset-0 source
                # (row g=1 is staged down from partition 32 by a tiny
                # scalar-issued SBUF->SBUF copy).
                abg = []
                for g in range(ng):
                    ab = abp.tile([P, EST], BF16, tag="ab")
                    if g == 0:
                        src_row = p_rows[0:1, :]
                    else:
                        pr1 = smp.tile([1, EST], BF16, tag="pr1")
                        nc.scalar.dma_start(out=pr1, in_=p_rows[32:33, :])
                        src_row = pr1
                    nc.gpsimd.partition_broadcast(out_ap=ab, in_ap=src_row)
                    abg.append(ab)
                for di in range(NDC):
                    for g in range(ng):
                        junk = junkp.tile([P, EST], BF16, tag="junk")
                        nc.vector.scalar_tensor_tensor(
                            out=junk,
                            in0=et[:, di, g * EST:(g + 1) * EST],
                            scalar=1.0,
                            in1=abg[g],
                            op0=ALU.mult,
                            op1=ALU.mult,
                            accum_out=cacc[:, di, r0 + g:r0 + g + 1],
                        )

            prev = None
            s0 = 0
            for t, w in enumerate(BATCH_WIDTHS[b]):
                et = encp.tile([P, NDC, ST], BF16, tag="et")
                nc.sync.dma_start(
                    out=et[:, :, 0:w],
                    in_=encT[:][b, :, :, s0:s0 + w].rearrange("di p s -> p di s"))

                ht = htp.tile([P, NKC, ST], BF16, tag="ht")
                for ki in range(NKC):
                    pre_ps = prep.tile([P, ST], F32, tag="pre")
                    for half in range(w // EST):
                        sl = slice(half * EST, (half + 1) * EST)
                        for di in range(NDC):
                            nc.tensor.matmul(
                                out=pre_ps[:, sl],
                                lhsT=w_sb[:, di, ki * P:(ki + 1) * P],
                                rhs=et[:, di, sl],
                                start=(di == 0),
                                stop=(di == NDC - 1),
                            )
                    # h^T = tanh(pre^T + c), one [128, w] ACT op per ki
                    nc.scalar.activation(
                        out=ht[:, ki, 0:w],
                        in_=pre_ps[:, 0:w],
                        func=AF.Tanh,
                        bias=c_sb[:, ki, b:b + 1],
                        scale=1.0,
                    )
                    # half-tile software pipelining: the previous tile's
                    # e/softmax/context chain slots in after this tile's
                    # first ki group, when its tanh inputs are ready but
                    # well before the PE would stall on them.
                    if ki == 0 and prev is not None:
                        softmax_ctx(*prev)
                        prev = None
                prev = (t, w, s0 // EST, et, ht)
                s0 += w
            softmax_ctx(*prev)

            # ---- finalize: ctx = (sum_s p*enc) / sum_s p ----
            lsum2 = smp.tile([P, 1], F32, tag="lsum2")
            nc.vector.reduce_sum(out=lsum2, in_=lparts, axis=mybir.AxisListType.X)
            # cross-partition total via gpsimd all-reduce: every partition
            # gets sum_s p directly (no gather DMAs, no re-broadcast)
            lsum = smp.tile([P, 1], F32, tag="lsum")
            nc.gpsimd.partition_all_reduce(
                out_ap=lsum, in_ap=lsum2, channels=P,
                reduce_op=bass_isa.ReduceOp.add,
            )
            rinvb = smp.tile([P, 1], F32, tag="rinvb")
            nc.vector.reciprocal(out=rinvb, in_=lsum)
            ctx_acc = ctxp.tile([P, NDC], F32, tag="ctx")
            nc.vector.reduce_sum(out=ctx_acc, in_=cacc, axis=mybir.AxisListType.X)
            nc.vector.tensor_scalar_mul(out=ctx_acc, in0=ctx_acc, scalar1=rinvb)
            nc.sync.dma_start(out=ctx_out[:][:, :, b], in_=ctx_acc)


_NC_CACHE = None


def _get_nc():
    global _NC_CACHE
    if _NC_CACHE is None:
        _NC_CACHE = build_bass()
    return _NC_CACHE


def _prep_core_inputs(hidden_state, encoder_outputs, W1, b1, w2, core):
    bf16 = ml_dtypes.bfloat16
    b0 = core * BPC
    enc = encoder_outputs[b0:b0 + BPC]                      # [BPC, S, D] f32
    # [b, d, s] -> [b, di, p, s] flat along s
    e = enc.transpose(0, 2, 1).reshape(BPC, NDC, P, S).astype(bf16)
    e = np.ascontiguousarray(e)
    w2cv = np.zeros((P, NKC, 16), dtype=np.float32)
    w2cv[:, :, 0] = w2.reshape(NKC, P).T
    return {
        "encT": e,
        "weT": np.ascontiguousarray(W1[:, :D].T.reshape(NDC, P, D)).astype(bf16),
        "cT": np.ascontiguousarray(
            (hidden_state[b0:b0 + BPC] @ W1[:, D:].T + b1).T.reshape(NKC, P, BPC)
            .transpose(1, 0, 2)),
        "w2c": w2cv.astype(bf16),
    }


def kernel(hidden_state, encoder_outputs, W1, b1, w2, _trace=False, _trace_kwargs=None):
    hidden_state = np.asarray(hidden_state, dtype=np.float32)
    encoder_outputs = np.asarray(encoder_outputs, dtype=np.float32)
    W1 = np.asarray(W1, dtype=np.float32)
    b1 = np.asarray(b1, dtype=np.float32)
    w2 = np.asarray(w2, dtype=np.float32)

    nc = _get_nc()
    in_maps = [
        _prep_core_inputs(hidden_state, encoder_outputs, W1, b1, w2, c)
        for c in range(NCORES)
    ]
    res = run_bass_kernel_spmd(
        nc, in_maps, list(range(NCORES)), trace=_trace,
        **(_trace_kwargs or {}),
    )
    out = np.empty((B, D), dtype=np.float32)
    for c in range(NCORES):
        r = res.results[c]["ctx"]                          # [p, di, b]
        out[c * BPC:(c + 1) * BPC] = r.transpose(2, 1, 0).reshape(BPC, D)
    if _trace:
        return out, res
    return out


# revision 14
# speedup vs baseline: 1.2082x; 1.0044x over previous
"""Bahdanau additive attention kernel for Trainium2 (8 NeuronCores).

Reference computation (B=32, S=4096, D=512):
    pre   = enc @ We.T + (hidden @ Wh.T + b1)[:, None, :]   # [B, S, D]
    h     = tanh(pre)
    e     = h @ w2                                          # [B, S]
    alpha = softmax(e, axis=1)
    ctx   = einsum('bs,bsd->bd', alpha, enc)                # [B, D]

Data-parallel over batch (4 batches per core); enc is re-laid-out on host
as [b, di, p, s] bf16 so the contraction dim d sits on SBUF partitions.

Everything stays bf16: fp8 anywhere upstream of alpha fails the accuracy
gate, because ctx is a softmax-weighted mean of zero-mean vectors - the
signal shrinks exactly as fast as independent noise averages out, so any
per-element quantization error lands on ctx at full relative strength
(measured: fp8 enc => 2.7e-2, fp8 main matmul => 2.2e-2 rel err).

Schedule (per core, vs. the naive version):
  - The tiny bias vector c = hidden @ Wh.T + b1 is computed on host
    (1 MFLOP) and rides the ACT bias operand, removing a 1 MB fp32
    weight DMA and 16 fp32 PE matmuls from the critical startup path.
  - PE warm-up burst on a memzero'd tile (no DMA dependency) brings the
    HAM clock gate to 8/8 while the first enc tile streams in.
  - e matmuls are column-group packed (tile_position=(0, 0)/(0, 32)):
    the two 512-wide e rows of an s-tile accumulate CONCURRENTLY on
    separate PE column groups / XBUS streams / PSUM banks, halving their
    PE cost; the zero-padded w2 column trick gives M=16 outputs of which
    row 0 is live.
  - Half-tile software pipelining: tile t's e/softmax/context chain is
    emitted after tile t+1's first ki group, so the in-order PE queue
    never waits on tanh, while the alpha round-trip latency stays ~2 us.
  - exp runs with fused row-sum accumulation (unnormalized streaming
    softmax: |e| < ~5 so no max pass); the per-batch row-sum partials
    are summed across partitions by a gpsimd partition_all_reduce and
    1/sum scales the final context once per batch.
  - alpha rows are replicated across partitions by gpsimd
    partition_broadcast COMPUTE ops (no DRAM round trip; broadcast APs
    must be canonical - full-tile dest, partition-0/offset-0 source, so
    the second e row is staged down from partition 32 by a tiny
    scalar-issued copy; sliced/offset APs silently misread on HW).
  - Strict DMA queue roles: sync carries ONLY enc tiles (the PE-feeding
    stream is never queued behind softmax-dependent writes); gpsimd
    issues NO DMAs at all, which removes its ~7 us SWDGE ring drain
    from the measured epilogue.
  - Variable s-tile widths: batch 0 opens with a 512-wide tile (halves
    the pipeline-fill DMA latency), the last batch closes with two
    512-wide tiles (halves the post-PE softmax/context drain).
"""

import sys

if "/opt/trn_rl_repo" not in sys.path:
    sys.path.insert(0, "/opt/trn_rl_repo")

from contextlib import ExitStack

import ml_dtypes
import numpy as np

import concourse.bass as bass
import concourse.bacc as bacc
from concourse import bass_isa
import concourse.tile as tile
from concourse import mybir
from concourse.bass_utils import run_bass_kernel_spmd

B, S, D = 32, 4096, 512
NCORES = 8
BPC = B // NCORES          # batches per core
P = 128                    # partitions
NDC = D // P               # d (contraction) chunks
NKC = D // P               # k (output channel) chunks
ST = 1024                  # max s-tile width (buffer sizing)
EST = 512                  # e-row granularity
NER = S // EST             # exp rows per batch
# s-tile widths per batch: a narrow first tile on batch 0 shortens the
# pipeline-fill DMA latency; narrow last tiles on the final batch shorten
# the softmax/context drain after the PE finishes.
BATCH_WIDTHS = [
    [512, 1024, 1024, 1024, 512],
    [1024, 1024, 1024, 1024],
    [1024, 1024, 1024, 1024],
    [1024, 1024, 1024, 512, 512],
]
NST = max(len(w) for w in BATCH_WIDTHS)

F32 = mybir.dt.float32
BF16 = mybir.dt.bfloat16
AF = mybir.ActivationFunctionType
ALU = mybir.AluOpType


def build_bass():
    nc = bacc.Bacc()

    encT = nc.declare_dram_parameter("encT", [BPC, NDC, P, S], BF16, isOutput=False)
    weT = nc.declare_dram_parameter("weT", [NDC, P, D], BF16, isOutput=False)
    cT = nc.declare_dram_parameter("cT", [P, NKC, BPC], F32, isOutput=False)
    # w2c[p, ki, m]: column 0 holds w2[ki*128+p], columns 1..15 zero
    w2c = nc.declare_dram_parameter("w2c", [P, NKC, 16], BF16, isOutput=False)
    ctx_out = nc.declare_dram_parameter("ctx", [P, NDC, BPC], F32, isOutput=True)

    with TileKernel(nc) as tk:
        tk.build(encT, weT, cT, w2c, ctx_out)
    nc.finalize()
    return nc


class TileKernel:
    def __init__(self, nc):
        self.nc = nc
        self.stack = ExitStack()
        self.tc = None

    def __enter__(self):
        self.tc = self.stack.enter_context(tile.TileContext(self.nc))
        return self

    def __exit__(self, *exc):
        return self.stack.__exit__(*exc)

    def build(self, encT, weT, cT, w2c, ctx_out):
        nc, tc, ctx = self.nc, self.tc, self.stack

        singles = ctx.enter_context(tc.tile_pool(name="singles", bufs=1))
        encp = ctx.enter_context(tc.tile_pool(name="encp", bufs=2 * NST))
        htp = ctx.enter_context(tc.tile_pool(name="htp", bufs=4))
        abp = ctx.enter_context(tc.tile_pool(name="abp", bufs=8))
        junkp = ctx.enter_context(tc.tile_pool(name="junkp", bufs=3))
        smp = ctx.enter_context(tc.tile_pool(name="smp", bufs=3))
        ctxp = ctx.enter_context(tc.tile_pool(name="ctxp", bufs=2))
        prep = ctx.enter_context(tc.tile_pool(name="prep", bufs=3, space="PSUM"))
        ecp = ctx.enter_context(tc.tile_pool(name="ecp", bufs=2, space="PSUM"))

        # ---- load constants ----
        # w_sb rides the sync queue ahead of the enc tiles; the other
        # constants go via the gpsimd queue so they don't delay enc tile 0.
        w_sb = singles.tile([P, NDC, D], BF16)
        nc.scalar.dma_start(out=w_sb, in_=weT[:].rearrange("di p k -> p di k"))
        w2_sb = singles.tile([P, NKC, 16], BF16)
        nc.scalar.dma_start(out=w2_sb, in_=w2c[:])
        c_sb = singles.tile([P, NKC, BPC], F32)
        nc.scalar.dma_start(out=c_sb, in_=cT[:])

        # ---- PE warm-up burst ----
        # ~6 us of dummy matmuls on a zeroed tile (no DMA dependency) so the
        # HAM clock gate reaches 8/8 before real work starts.
        wz = singles.tile([P, D], BF16)
        nc.vector.memzero(wz)
        wpre = prep.tile([P, ST], F32, tag="pre")
        for i in range(11):
            nc.tensor.matmul(
                out=wpre[:, 0:D], lhsT=wz[:, 0:P], rhs=wz,
                start=True, stop=True,
            )
        wjunk = singles.tile([P, 1], F32)
        nc.vector.tensor_copy(out=wjunk, in_=wpre[:, 0:1])

        # ---- main per-batch pipeline ----
        # Softmax runs UNNORMALIZED and streamed: e is bounded (|e| < ~5)
        # so exp needs no max pass; exp(e) rows are computed as soon as the
        # e matmuls land in PSUM, round-trip through DRAM (bf16) to
        # partition-broadcast, and feed the context accumulation while the
        # PE works on later s-tiles. The 1/sum(exp) normalization is
        # applied once to the final [128, NDC] context.
        for b in range(BPC):
            nbt = len(BATCH_WIDTHS[b])
            lparts = smp.tile([P, NST], F32, tag="lparts")
            nc.vector.memzero(lparts)
            cacc = ctxp.tile([P, NDC, nbt], F32, tag="cacc")

            def softmax_ctx(t, w, r0, et, ht):
                ng = w // EST
                # e rows, packed on PE column groups 0/32: the (up to) two
                # accumulation chains run concurrently (separate XBUS
                # streams, separate PSUM banks + partition ranges). Emitted
                # one s-tile behind the main matmuls so the PE never waits
                # on tanh.
                e_ps0 = ecp.tile([16, EST], F32, tag="ec")
                e_ps1 = ecp.tile([48, EST], F32, tag="ec")
                e_pss = [e_ps0, e_ps1]
                for ki in range(NKC):
                    for g in range(ng):
                        sl = slice(g * EST, (g + 1) * EST)
                        nc.tensor.matmul(
                            out=e_pss[g][32 * g:32 * g + 16, :],
                            lhsT=w2_sb[:, ki, :],
                            rhs=ht[:, ki, sl],
                            start=(ki == 0),
                            stop=(ki == NKC - 1),
                            tile_position=(0, 32 * g),
                        )
                # p = exp(e) per row with the row-sum fused; bf16 rows go
                # out through DRAM so DMA can replicate across partitions.
                p_rows = smp.tile([48, EST], BF16, tag="prow")
                for g in range(ng):
                    nc.scalar.activation(
                        out=p_rows[32 * g:32 * g + 1, :],
                        in_=e_pss[g][32 * g:32 * g + 1, :],
                        func=AF.Exp, bias=0.0, scale=1.0,
                        accum_out=lparts[32 * g:32 * g + 1, t:t + 1],
                    )
                # replicate the p rows across partitions with gpsimd
                # partition_broadcast compute ops: no DRAM round trip, and
                # gpsimd stays DMA-free so its SWDGE drain vanishes from
                # the epilogue. The broadcast gets canonical APs only:
                # full-tile outputs, and a partition-0/offset-0 source
                # (row g=1 is staged down from partition 32 by a tiny
                # scalar-issued SBUF->SBUF copy).
                ab = abp.tile([P, ST], BF16, tag="ab")
                for g in range(ng):
                    if g == 0:
                        src_row = p_rows[0:1, :]
                    else:
                        pr1 = smp.tile([1, EST], BF16, tag="pr1")
                        nc.scalar.dma_start(out=pr1, in_=p_rows[32:33, :])
                        src_row = pr1
                    nc.gpsimd.partition_broadcast(
                        out_ap=ab[:, g * EST:(g + 1) * EST], in_ap=src_row)
                for di in range(NDC):
                    junk = junkp.tile([P, ST], BF16, tag="junk")
                    nc.vector.scalar_tensor_tensor(
                        out=junk[:, 0:w],
                        in0=et[:, di, 0:w],
                        scalar=1.0,
                        in1=ab[:, 0:w],
                        op0=ALU.mult,
                        op1=ALU.mult,
                        accum_out=cacc[:, di, t:t + 1],
                    )

            prev = None
            s0 = 0
            for t, w in enumerate(BATCH_WIDTHS[b]):
                et = encp.tile([P, NDC, ST], BF16, tag="et")
                nc.sync.dma_start(
                    out=et[:, :, 0:w],
                    in_=encT[:][b, :, :, s0:s0 + w].rearrange("di p s -> p di s"))

                ht = htp.tile([P, NKC, ST], BF16, tag="ht")
                for ki in range(NKC):
                    pre_ps = prep.tile([P, ST], F32, tag="pre")
                    for half in range(w // EST):
                        sl = slice(half * EST, (half + 1) * EST)
                        for di in range(NDC):
                            nc.tensor.matmul(
                                out=pre_ps[:, sl],
                                lhsT=w_sb[:, di, ki * P:(ki + 1) * P],
                                rhs=et[:, di, sl],
                                start=(di == 0),
                                stop=(di == NDC - 1),
                            )
                    # h^T = tanh(pre^T + c), one [128, w] ACT op per ki
                    nc.scalar.activation(
                        out=ht[:, ki, 0:w],
                        in_=pre_ps[:, 0:w],
                        func=AF.Tanh,
                        bias=c_sb[:, ki, b:b + 1],
                        scale=1.0,
                    )
                    # half-tile software pipelining: the previous tile's
                    # e/softmax/context chain slots in after this tile's
                    # first ki group, when its tanh inputs are ready but
                    # well before the PE would stall on them.
                    if ki == 0 and prev is not None:
                        softmax_ctx(*prev)
                        prev = None
                prev = (t, w, s0 // EST, et, ht)
                s0 += w
            softmax_ctx(*prev)

            # ---- finalize: ctx = (sum_s p*enc) / sum_s p ----
            lsum2 = smp.tile([P, 1], F32, tag="lsum2")
            nc.vector.reduce_sum(out=lsum2, in_=lparts, axis=mybir.AxisListType.X)
            # cross-partition total via gpsimd all-reduce: every partition
            # gets sum_s p directly (no gather DMAs, no re-broadcast)
            lsum = smp.tile([P, 1], F32, tag="lsum")
            nc.gpsimd.partition_all_reduce(
                out_ap=lsum, in_ap=lsum2, channels=P,
                reduce_op=bass_isa.ReduceOp.add,
            )
            rinvb = smp.tile([P, 1], F32, tag="rinvb")
            nc.vector.reciprocal(out=rinvb, in_=lsum)
            ctx_acc = ctxp.tile([P, NDC], F32, tag="ctx")
            nc.vector.reduce_sum(out=ctx_acc, in_=cacc, axis=mybir.AxisListType.X)
            nc.vector.tensor_scalar_mul(out=ctx_acc, in0=ctx_acc, scalar1=rinvb)
            nc.sync.dma_start(out=ctx_out[:][:, :, b], in_=ctx_acc)


_NC_CACHE = None


def _get_nc():
    global _NC_CACHE
    if _NC_CACHE is None:
        _NC_CACHE = build_bass()
    return _NC_CACHE


def _prep_core_inputs(hidden_state, encoder_outputs, W1, b1, w2, core):
    bf16 = ml_dtypes.bfloat16
    b0 = core * BPC
    enc = encoder_outputs[b0:b0 + BPC]                      # [BPC, S, D] f32
    # [b, d, s] -> [b, di, p, s] flat along s
    e = enc.transpose(0, 2, 1).reshape(BPC, NDC, P, S).astype(bf16)
    e = np.ascontiguousarray(e)
    w2cv = np.zeros((P, NKC, 16), dtype=np.float32)
    w2cv[:, :, 0] = w2.reshape(NKC, P).T
    return {
        "encT": e,
        "weT": np.ascontiguousarray(W1[:, :D].T.reshape(NDC, P, D)).astype(bf16),
        "cT": np.ascontiguousarray(
            (hidden_state[b0:b0 + BPC] @ W1[:, D:].T + b1).T.reshape(NKC, P, BPC)
            .transpose(1, 0, 2)),
        "w2c": w2cv.astype(bf16),
    }


def kernel(hidden_state, encoder_outputs, W1, b1, w2, _trace=False, _trace_kwargs=None):
    hidden_state = np.asarray(hidden_state, dtype=np.float32)
    encoder_outputs = np.asarray(encoder_outputs, dtype=np.float32)
    W1 = np.asarray(W1, dtype=np.float32)
    b1 = np.asarray(b1, dtype=np.float32)
    w2 = np.asarray(w2, dtype=np.float32)

    nc = _get_nc()
    in_maps = [
        _prep_core_inputs(hidden_state, encoder_outputs, W1, b1, w2, c)
        for c in range(NCORES)
    ]
    res = run_bass_kernel_spmd(
        nc, in_maps, list(range(NCORES)), trace=_trace,
        **(_trace_kwargs or {}),
    )
    out = np.empty((B, D), dtype=np.float32)
    for c in range(NCORES):
        r = res.results[c]["ctx"]                          # [p, di, b]
        out[c * BPC:(c + 1) * BPC] = r.transpose(2, 1, 0).reshape(BPC, D)
    if _trace:
        return out, res
    return out
